# revision 24
# baseline (speedup 1.0000x reference)
"""Trainium2 Bass kernel for nn_FCLSTM (3-tower cross-fed bidirectional 2-layer
stepwise LSTM + mean-pool + linear decoder).

Strategy (8 NeuronCores, one chip):
  * Model-parallel over the gate/hidden dimension: core k owns rows
    [64k:64k+64] of every gate (i,f,g,o) of every cell (3 towers x 2 layers x
    2 directions = 12 cells). All weights SBUF-resident in bf16
    (~10.4 MB/core). Batch B=128 rides the matmul free dim.
  * Matmul orientation: out(B=128 partitions, gates<=512 free) =
    lhsT(K=128 input-feature rows, B).T @ rhs(K, gate cols). Activations are
    the stationary operand (feature-major; x is pre-transposed on host),
    weights are the moving operand.
  * Per timestep, two 8-core AllGathers exchange the freshly computed 64-row
    h chunks so every core has the full h for the next contraction
    (layer0 -> AG1 -> layer1 -> AG2 -> next step). Biases are folded into the
    matmuls via a constant ones-row in the stationary tiles.
  * The decoder (B,1536)@(1536,1) runs on host in fp32 from the per-core
    h_l1_bwd running sums.

kernel(**inputs) accepts the FULL unsharded inputs and returns the full
(128, 1) float32 output.
"""
import contextlib
import numpy as np
import ml_dtypes

import concourse.bass as bass
import concourse.tile as tile
from concourse import bacc, mybir
from concourse.bass_utils import run_bass_kernel_spmd

# ---------------- problem constants (hardcoded per spec) ----------------
B = 128
S = 128
H = 512
NC_ = 8           # cores
CH = H // NC_     # 64: per-core hidden chunk per gate
DX = {"t": 300, "v": 512, "a": 128}
TOWERS = ("t", "v", "a")
FEEDS = {"t": ("v", "a"), "v": ("t", "a"), "a": ("t", "v")}

BF16 = mybir.dt.bfloat16
F32 = mybir.dt.float32

# x stationary-tile layout: per tower group, ceil((Dx+1)/128) tiles of 128 rows
# (zero padded), with the bias/ones row right after the x rows.
XT_TILES = {"t": 3, "v": 5, "a": 2}         # 300+1 -> 3, 512+1 -> 5, 128+1 -> 2
XT_OFF = {"t": 0, "v": 3, "a": 8}           # tile offset in the 10-tile stack
NXT = 10
ONES_POS = {"t": (2, 44), "v": (4, 0), "a": (1, 0)}   # (tile-in-group, row)
# l1 bias rides tower-t's ones tile (global x-tile 2, row 44)
L1_BIAS_TILE, L1_BIAS_ROW = 2, 44

PAIRS = [("t", 0), ("v", 0), ("a", 0), ("t", 1), ("v", 1), ("a", 1)]
HKT = H // 128    # 4 K-tiles per 512-row h


def _wsh_layout():
    """Block indices in the shared-weight stack for each pair."""
    off = {}
    n = 0
    for tw, l in PAIRS:
        if l == 0:
            nx = XT_TILES[tw]
            off[(tw, l)] = {"x": n, "f1": n + nx, "f2": n + nx + HKT}
            n += nx + 2 * HKT
        else:
            off[(tw, l)] = {"hf": n, "hb": n + HKT, "bias": n + 2 * HKT}
            n += 2 * HKT + 1
    return off, n


WSH_OFF, NWSH = _wsh_layout()   # NWSH = 61
CELLS = [(tw, l, d) for (tw, l) in PAIRS for d in (0, 1)]   # 12, pair-major


# ---------------- host-side packing ----------------

def _to_bf16(x):
    return np.ascontiguousarray(np.asarray(x, np.float32)).astype(ml_dtypes.bfloat16)


def _gate_rows(W, k):
    """Rows of pytorch-layout (4H, in) W for core k, column order [i f o g].
    Returns (in, 256) transposed."""
    i = W[0 * H + k * CH: 0 * H + k * CH + CH]
    f = W[1 * H + k * CH: 1 * H + k * CH + CH]
    g = W[2 * H + k * CH: 2 * H + k * CH + CH]
    o = W[3 * H + k * CH: 3 * H + k * CH + CH]
    return np.concatenate([i, f, o, g], axis=0).T.astype(np.float32)


def _gate_bias(b_ih, b_hh, k):
    b = np.asarray(b_ih, np.float32) + np.asarray(b_hh, np.float32)
    i = b[0 * H + k * CH: 0 * H + k * CH + CH]
    f = b[1 * H + k * CH: 1 * H + k * CH + CH]
    g = b[2 * H + k * CH: 2 * H + k * CH + CH]
    o = b[3 * H + k * CH: 3 * H + k * CH + CH]
    return np.concatenate([i, f, o, g])


def prep_in_maps(t_input, v_input, a_input, params_t, params_v, params_a,
                 dec_W, dec_b, n_steps=S):
    """Builds the 8 per-core input maps (numpy, bf16-packed)."""
    xs = {"t": np.asarray(t_input, np.float32),
          "v": np.asarray(v_input, np.float32),
          "a": np.asarray(a_input, np.float32)}
    params = {"t": params_t, "v": params_v, "a": params_a}

    # x_T: (S, NXT, 128, 128) bf16, replicated to all cores.
    x_T = np.zeros((n_steps, NXT, 128, 128), np.float32)
    for tw in TOWERS:
        d = DX[tw]
        g0 = XT_OFF[tw]
        xt = xs[tw][:, :n_steps, :].transpose(1, 2, 0)    # (S, D, B)
        full, rem = d // 128, d % 128
        for t_ in range(full):
            x_T[:, g0 + t_, :, :] = xt[:, t_ * 128:(t_ + 1) * 128, :]
        if rem:
            x_T[:, g0 + full, :rem, :] = xt[:, full * 128:, :]
        ot, orow = ONES_POS[tw]
        x_T[:, g0 + ot, orow, :] = 1.0
    x_T = _to_bf16(x_T)

    def cell_params(tw, l, d_):
        return params[tw][l][d_]   # (W_ih, W_hh, b_ih, b_hh)

    in_maps = []
    for k in range(NC_):
        wsh = np.zeros((NWSH, 128, 512), np.float32)
        for tw, l in PAIRS:
            Wf = cell_params(tw, l, 0)
            Wb = cell_params(tw, l, 1)
            cols_f = _gate_rows(np.asarray(Wf[0], np.float32), k)   # (in, 256)
            cols_b = _gate_rows(np.asarray(Wb[0], np.float32), k)
            bias = np.concatenate([_gate_bias(Wf[2], Wf[3], k),
                                   _gate_bias(Wb[2], Wb[3], k)])     # (512,)
            cols = np.concatenate([cols_f, cols_b], axis=1)          # (in, 512)
            o = WSH_OFF[(tw, l)]
            if l == 0:
                d = DX[tw]
                for t_ in range(XT_TILES[tw]):
                    lo = t_ * 128
                    hi = min(d, lo + 128)
                    if hi > lo:
                        wsh[o["x"] + t_, 0:hi - lo, :] = cols[lo:hi, :]
                ot, orow = ONES_POS[tw]
                wsh[o["x"] + ot, orow, :] = bias
                for fi, key in ((0, "f1"), (1, "f2")):
                    base = d + fi * 512
                    for t_ in range(HKT):
                        wsh[o[key] + t_, :, :] = cols[base + t_ * 128: base + (t_ + 1) * 128, :]
            else:
                for t_ in range(HKT):
                    wsh[o["hf"] + t_, :, :] = cols[t_ * 128:(t_ + 1) * 128, :]
                    wsh[o["hb"] + t_, :, :] = cols[512 + t_ * 128: 512 + (t_ + 1) * 128, :]
                wsh[o["bias"], L1_BIAS_ROW, :] = bias
        whh = np.zeros((12, HKT, 128, 256), np.float32)
        for ci, (tw, l, d_) in enumerate(CELLS):
            Wc = cell_params(tw, l, d_)
            cols = _gate_rows(np.asarray(Wc[1], np.float32), k)      # (512, 256)
            for t_ in range(HKT):
                whh[ci, t_, :, :] = cols[t_ * 128:(t_ + 1) * 128, :]
        in_maps.append({"x_T": x_T, "wsh": _to_bf16(wsh), "whh": _to_bf16(whh)})
    return in_maps


def host_decode(results, dec_W, dec_b, n_steps=S):
    """results: list of 8 per-core dicts with 'st_out' (3, 128, 64) f32."""
    hidden = np.zeros((B, 3 * H), np.float32)
    for k in range(NC_):
        st = np.asarray(results[k]["st_out"], np.float32)   # (3, B, 64)
        for ti in range(3):
            hidden[:, ti * H + k * CH: ti * H + (k + 1) * CH] = st[ti]
    hidden *= 1.0 / n_steps
    dec_W = np.asarray(dec_W, np.float32)
    dec_b = np.asarray(dec_b, np.float32)
    return hidden @ dec_W.T + dec_b


# ---------------- bass kernel builder ----------------

def build_kernel(n_steps=S, cc_mode="real", with_ew=True, nd=NC_):
    nc = bacc.Bacc("TRN2", target_bir_lowering=False, debug=False,
                   enable_asserts=False, num_devices=nd)
    x_dram = nc.dram_tensor("x_T", (n_steps, NXT, 128, 128), BF16,
                            kind="ExternalInput").ap()
    wsh_dram = nc.dram_tensor("wsh", (NWSH, 128, 512), BF16,
                              kind="ExternalInput").ap()
    whh_dram = nc.dram_tensor("whh", (12, HKT, 128, 256), BF16,
                              kind="ExternalInput").ap()
    st_dram = nc.dram_tensor("st_out", (3, 128, CH), F32,
                             kind="ExternalOutput").ap()

    RG = [list(range(NC_))]

    with tile.TileContext(nc) as tc:
        with contextlib.ExitStack() as ctx:
            persist = ctx.enter_context(tc.tile_pool(name="persist", bufs=1))
            xp = ctx.enter_context(tc.tile_pool(name="xp", bufs=2))
            hg = ctx.enter_context(tc.tile_pool(name="hg", bufs=2))
            actp = ctx.enter_context(tc.tile_pool(name="actp", bufs=2))
            tmpp = ctx.enter_context(tc.tile_pool(name="tmpp", bufs=4))
            cp = ctx.enter_context(tc.tile_pool(name="cp", bufs=2))
            hp = ctx.enter_context(tc.tile_pool(name="hp", bufs=2))
            htp = ctx.enter_context(tc.tile_pool(name="htp", bufs=2))
            stp = ctx.enter_context(tc.tile_pool(name="stp", bufs=2))
            pg = ctx.enter_context(tc.tile_pool(name="pg", bufs=1, space="PSUM"))
            dramp = ctx.enter_context(tc.tile_pool(name="dramp", bufs=2, space="DRAM"))

            # ---- weight preload
            wsh_sb = persist.tile([128, NWSH, 512], BF16)
            nc.sync.dma_start(wsh_sb[:], wsh_dram.rearrange("w p n -> p w n"))
            whh_sb = persist.tile([128, 12 * HKT, 256], BF16)
            nc.sync.dma_start(whh_sb[:], whh_dram.rearrange("c k p n -> p (c k) n"))

            # persistent state handles, rotated via pool tags
            c_tiles = {}    # pair -> tile (128, 128) f32  [fwd 64 | bwd 64]
            st_tiles = {}   # tower-idx -> tile (128, 64) f32
            hl_all = {}     # layer -> gathered-h tile (128, 2, 3, HKT, 128) bf16
            hl_tiles = {}   # (pair, dir) -> (tile, j) accessor base

            ident = persist.tile([128, 128], BF16)
            from concourse.masks import make_identity
            make_identity(nc, ident[:])

            def load_x(s):
                t_ = xp.tile([128, NXT, 128], BF16, tag="x")
                nc.sync.dma_start(t_[:], x_dram[s].rearrange("g p b -> p g b"))
                return t_

            def pair_idx(tw, l):
                return PAIRS.index((tw, l))

            def mm(gt_ap, lhsT, rhs, start, stop):
                nc.tensor.matmul(gt_ap, lhsT, rhs, start=start, stop=stop)

            def emit_shared(gt, tw, l, x_sb_, first, last):
                """Shared-K matmuls (x part for l0 / h_l0 part + bias for l1)."""
                o = WSH_OFF[(tw, l)]
                if l == 0:
                    g0 = XT_OFF[tw]
                    nx = XT_TILES[tw]
                    for t_ in range(nx):
                        mm(gt[:], x_sb_[:, g0 + t_, :], wsh_sb[:, o["x"] + t_, :],
                           start=first and t_ == 0, stop=last and t_ == nx - 1)
                else:
                    for d_, key in ((0, "hf"), (1, "hb")):
                        src, j = hl_tiles[(pair_idx(tw, 0), d_)]
                        for t_ in range(HKT):
                            mm(gt[:], src[:, t_, d_, j, :], wsh_sb[:, o[key] + t_, :],
                               start=first and d_ == 0 and t_ == 0, stop=False)
                    mm(gt[:], x_sb_[:, L1_BIAS_TILE, :], wsh_sb[:, o["bias"], :],
                       start=False, stop=last)

            def emit_feeds(gt, tw, last):
                o = WSH_OFF[(tw, 0)]
                f1, f2 = FEEDS[tw]
                for ki, (key, ftw) in enumerate((("f1", f1), ("f2", f2))):
                    src, j = hl_tiles[(pair_idx(ftw, 1), 1)]  # l1 bwd of feed tower
                    for t_ in range(HKT):
                        mm(gt[:], src[:, t_, 1, j, :], wsh_sb[:, o[key] + t_, :],
                           start=False, stop=last and ki == 1 and t_ == HKT - 1)

            def emit_whh(gt, tw, l, first):
                p = pair_idx(tw, l)
                for d_ in (0, 1):
                    ci = p * 2 + d_
                    src, j = hl_tiles[(p, d_)]
                    for t_ in range(HKT):
                        mm(gt[:, d_ * 256:(d_ + 1) * 256], src[:, t_, d_, j, :],
                           whh_sb[:, ci * HKT + t_, :],
                           start=first and d_ == 0 and t_ == 0, stop=False)

            def emit_elementwise(s, tw, l, gt):
                """gates (128, 512) psum [fwd ifog | bwd ifog] -> h_pair bf16;
                updates c and (for l1) st."""
                p = pair_idx(tw, l)
                if not with_ew:
                    h_pair = hp.tile([128, 128], BF16, tag=f"h{p}")
                    nc.vector.tensor_copy(h_pair[:], gt[:, 0:128])
                    if l == 1:
                        ti = TOWERS.index(tw)
                        st_new = stp.tile([128, 64], F32, tag=f"st{ti}")
                        nc.vector.tensor_copy(st_new[:], gt[:, 128:192])
                        st_tiles[ti] = st_new
                    return h_pair
                acts = actp.tile([128, 512], F32, tag=f"acts{p}")
                gtv = gt[:].rearrange("p (d n) -> p d n", d=2)
                av = acts[:].rearrange("p (d n) -> p d n", d=2)
                nc.scalar.activation(av[:, :, 0:192], gtv[:, :, 0:192],
                                     mybir.ActivationFunctionType.Sigmoid)
                nc.scalar.activation(av[:, :, 192:256], gtv[:, :, 192:256],
                                     mybir.ActivationFunctionType.Tanh)
                i_ap = av[:, :, 0:64]
                f_ap = av[:, :, 64:128]
                o_ap = av[:, :, 128:192]
                g_ap = av[:, :, 192:256]
                ig = tmpp.tile([128, 128], F32, tag="ig")
                igv = ig[:].rearrange("p (d n) -> p d n", d=2)
                nc.vector.tensor_mul(igv, i_ap, g_ap)
                c_new = cp.tile([128, 128], F32, tag=f"c{p}")
                cv = c_new[:].rearrange("p (d n) -> p d n", d=2)
                if s == 0:
                    nc.vector.tensor_copy(c_new[:], ig[:])
                else:
                    c_old = c_tiles[p]
                    nc.vector.tensor_mul(cv, f_ap,
                                         c_old[:].rearrange("p (d n) -> p d n", d=2))
                    nc.vector.tensor_add(c_new[:], c_new[:], ig[:])
                c_tiles[p] = c_new
                tanh_c = tmpp.tile([128, 128], F32, tag="tanh_c")
                nc.scalar.activation(tanh_c[:], c_new[:],
                                     mybir.ActivationFunctionType.Tanh)
                h_pair = hp.tile([128, 128], BF16, tag=f"h{p}")
                nc.vector.tensor_mul(h_pair[:].rearrange("p (d n) -> p d n", d=2),
                                     o_ap, tanh_c[:].rearrange("p (d n) -> p d n", d=2))
                if l == 1:
                    # st += h_bwd chunk, in fp32 (recompute the bwd mul in f32)
                    ti = TOWERS.index(tw)
                    h64 = tmpp.tile([128, 64], F32, tag="h64")
                    nc.vector.tensor_mul(h64[:], av[:, 1, 128:192],
                                         tanh_c[:, 64:128])
                    st_new = stp.tile([128, 64], F32, tag=f"st{ti}")
                    if s == 0:
                        nc.vector.tensor_copy(st_new[:], h64[:])
                    else:
                        nc.vector.tensor_add(st_new[:], st_tiles[ti][:], h64[:])
                    st_tiles[ti] = st_new
                return h_pair

            def emit_gather(l, h_pairs):
                """Transpose 3 h_pair tiles, AllGather, land per-(pair,dir)
                stationary tiles into hl_tiles."""
                ht = htp.tile([128, 3, 128], BF16, tag=f"ht{l}")
                for j in range(3):
                    tr = pg.tile([128, 128], BF16, tag="tr")
                    nc.tensor.transpose(tr[:], h_pairs[j][:], ident[:])
                    nc.vector.tensor_copy(ht[:, j, :], tr[:])
                # agin rows land in ht's row order: row p = (dir, sub), so the
                # gathered buffer is (rank, d, sub, pair, b) — readback needs
                # only 3-dim APs then.
                agin = dramp.tile([128, 3, 128], BF16, tag=f"agin{l}")
                nc.sync.dma_start(agin[:], ht[:])
                agout = dramp.tile([NC_, 128, 3, 128], BF16, tag=f"agout{l}")
                if cc_mode == "real":
                    nc.gpsimd.collective_compute(
                        "AllGather", mybir.AluOpType.bypass, replica_groups=RG,
                        ins=[agin.opt()], outs=[agout.opt()])
                else:
                    for r in range(NC_):
                        nc.sync.dma_start(agout[r], agin[:])
                hla = hg.tile([128, HKT, 2, 3, 128], BF16, tag=f"hla{l}")
                # gathered rank r = 2*kt + phi owns feature rows
                # [128*kt + 64*phi, +64) of each (pair, dir); view the bounce
                # through the tile AP so Tile tracks the read.
                agv = agout[:].rearrange("(kt phi) (dd sub) g b -> phi dd sub kt (g b)",
                                         phi=2, dd=2)
                for phi in (0, 1):
                    for d_ in (0, 1):
                        nc.sync.dma_start(
                            hla[64 * phi:64 * phi + 64, :, d_, :, :],
                            agv[phi, d_])
                hl_all[l] = hla
                for j in range(3):
                    p = 3 * l + j
                    for d_ in (0, 1):
                        hl_tiles[(p, d_)] = (hla, j)

            # ---------------- main loop ----------------
            x_sb = load_x(0)
            for s in range(n_steps):
                x_next = load_x(s + 1) if s + 1 < n_steps else None
                # ---- layer 0: x-part + recurrent first (ready during AG2(s-1)),
                # cross-feeds last (they wait on AG2(s-1)).
                gts = {}
                for tw in TOWERS:
                    gt = pg.tile([128, 512], F32, tag=f"g{pair_idx(tw, 0)}")
                    gts[tw] = gt
                    emit_shared(gt, tw, 0, x_sb, first=True, last=(s == 0))
                    if s > 0:
                        emit_whh(gt, tw, 0, first=False)
                if s > 0:
                    for tw in TOWERS:
                        emit_feeds(gts[tw], tw, last=True)
                h_l0 = [emit_elementwise(s, tw, 0, gts[tw]) for tw in TOWERS]
                # ---- layer 1 recurrent part first: depends only on AG2(s-1),
                # so PE chews it while the l0 elementwise + AG1 are in flight.
                gts1 = {}
                for tw in TOWERS:
                    gt = pg.tile([128, 512], F32, tag=f"g{pair_idx(tw, 1)}")
                    gts1[tw] = gt
                    if s > 0:
                        emit_whh(gt, tw, 1, first=True)
                emit_gather(0, h_l0)
                for tw in TOWERS:
                    emit_shared(gts1[tw], tw, 1, x_sb, first=(s == 0), last=True)
                h_l1 = [emit_elementwise(s, tw, 1, gts1[tw]) for tw in TOWERS]
                if s + 1 < n_steps:
                    emit_gather(1, h_l1)
                if x_next is not None:
                    x_sb = x_next

            # ---- write out st sums
            for ti in range(3):
                nc.sync.dma_start(st_dram[ti], st_tiles[ti][:])

    nc.compile()
    return nc


# ---------------- public entry ----------------
_CACHED = {}


def _get_kernel(n_steps=S):
    if n_steps not in _CACHED:
        _CACHED[n_steps] = build_kernel(n_steps)
    return _CACHED[n_steps]


def run_on_hw(inputs, n_steps=S):
    nc = _get_kernel(n_steps)
    in_maps = prep_in_maps(**inputs, n_steps=n_steps)
    res = run_bass_kernel_spmd(nc, in_maps, core_ids=list(range(NC_)))
    return host_decode(res.results, inputs["dec_W"], inputs["dec_b"],
                       n_steps=n_steps).astype(np.float32)


def kernel(t_input, v_input, a_input, params_t, params_v, params_a, dec_W, dec_b):
    return run_on_hw(dict(t_input=t_input, v_input=v_input, a_input=a_input,
                          params_t=params_t, params_v=params_v, params_a=params_a,
                          dec_W=dec_W, dec_b=dec_b), n_steps=S)


# revision 28
# speedup vs baseline: 1.0327x; 1.0327x over previous
"""Trainium2 Bass kernel for nn_FCLSTM (3-tower cross-fed bidirectional 2-layer
stepwise LSTM + mean-pool + linear decoder).

Strategy (8 NeuronCores, one chip):
  * Model-parallel over the gate/hidden dimension: core k owns rows
    [64k:64k+64] of every gate (i,f,g,o) of every cell (3 towers x 2 layers x
    2 directions = 12 cells). All weights SBUF-resident in bf16
    (~10.4 MB/core). Batch B=128 rides the matmul free dim.
  * Matmul orientation: out(B=128 partitions, gates<=512 free) =
    lhsT(K=128 input-feature rows, B).T @ rhs(K, gate cols). Activations are
    the stationary operand (feature-major; x is pre-transposed on host),
    weights are the moving operand.
  * Per timestep, two 8-core AllGathers exchange the freshly computed 64-row
    h chunks so every core has the full h for the next contraction
    (layer0 -> AG1 -> layer1 -> AG2 -> next step). Biases are folded into the
    matmuls via a constant ones-row in the stationary tiles.
  * The decoder (B,1536)@(1536,1) runs on host in fp32 from the per-core
    h_l1_bwd running sums.

kernel(**inputs) accepts the FULL unsharded inputs and returns the full
(128, 1) float32 output.
"""
import contextlib
import numpy as np
import ml_dtypes

import concourse.bass as bass
import concourse.tile as tile
from concourse import bacc, mybir
from concourse.bass_utils import run_bass_kernel_spmd

# ---------------- problem constants (hardcoded per spec) ----------------
B = 128
S = 128
H = 512
NC_ = 8           # cores
CH = H // NC_     # 64: per-core hidden chunk per gate
DX = {"t": 300, "v": 512, "a": 128}
TOWERS = ("t", "v", "a")
FEEDS = {"t": ("v", "a"), "v": ("t", "a"), "a": ("t", "v")}

BF16 = mybir.dt.bfloat16
F32 = mybir.dt.float32

# x stationary-tile layout: per tower group, ceil((Dx+1)/128) tiles of 128 rows
# (zero padded), with the bias/ones row right after the x rows.
XT_TILES = {"t": 3, "v": 5, "a": 2}         # 300+1 -> 3, 512+1 -> 5, 128+1 -> 2
XT_OFF = {"t": 0, "v": 3, "a": 8}           # tile offset in the 10-tile stack
NXT = 10
ONES_POS = {"t": (2, 44), "v": (4, 0), "a": (1, 0)}   # (tile-in-group, row)
# l1 bias rides tower-t's ones tile (global x-tile 2, row 44)
L1_BIAS_TILE, L1_BIAS_ROW = 2, 44

PAIRS = [("t", 0), ("v", 0), ("a", 0), ("t", 1), ("v", 1), ("a", 1)]
HKT = H // 128    # 4 K-tiles per 512-row h


def _wsh_layout():
    """Block indices in the shared-weight stack for each pair."""
    off = {}
    n = 0
    for tw, l in PAIRS:
        if l == 0:
            nx = XT_TILES[tw]
            off[(tw, l)] = {"x": n, "f1": n + nx, "f2": n + nx + HKT}
            n += nx + 2 * HKT
        else:
            off[(tw, l)] = {"hf": n, "hb": n + HKT, "bias": n + 2 * HKT}
            n += 2 * HKT + 1
    return off, n


WSH_OFF, NWSH = _wsh_layout()   # NWSH = 61
CELLS = [(tw, l, d) for (tw, l) in PAIRS for d in (0, 1)]   # 12, pair-major


# ---------------- host-side packing ----------------

def _to_bf16(x):
    return np.ascontiguousarray(np.asarray(x, np.float32)).astype(ml_dtypes.bfloat16)


def _gate_rows(W, k):
    """Rows of pytorch-layout (4H, in) W for core k, column order [i f o g].
    Returns (in, 256) transposed."""
    i = W[0 * H + k * CH: 0 * H + k * CH + CH]
    f = W[1 * H + k * CH: 1 * H + k * CH + CH]
    g = W[2 * H + k * CH: 2 * H + k * CH + CH]
    o = W[3 * H + k * CH: 3 * H + k * CH + CH]
    return np.concatenate([i, f, o, g], axis=0).T.astype(np.float32)


def _gate_bias(b_ih, b_hh, k):
    b = np.asarray(b_ih, np.float32) + np.asarray(b_hh, np.float32)
    i = b[0 * H + k * CH: 0 * H + k * CH + CH]
    f = b[1 * H + k * CH: 1 * H + k * CH + CH]
    g = b[2 * H + k * CH: 2 * H + k * CH + CH]
    o = b[3 * H + k * CH: 3 * H + k * CH + CH]
    return np.concatenate([i, f, o, g])


def prep_in_maps(t_input, v_input, a_input, params_t, params_v, params_a,
                 dec_W, dec_b, n_steps=S):
    """Builds the 8 per-core input maps (numpy, bf16-packed)."""
    xs = {"t": np.asarray(t_input, np.float32),
          "v": np.asarray(v_input, np.float32),
          "a": np.asarray(a_input, np.float32)}
    params = {"t": params_t, "v": params_v, "a": params_a}

    # x_T: (S, NXT, 128, 128) bf16, replicated to all cores.
    x_T = np.zeros((n_steps, NXT, 128, 128), np.float32)
    for tw in TOWERS:
        d = DX[tw]
        g0 = XT_OFF[tw]
        xt = xs[tw][:, :n_steps, :].transpose(1, 2, 0)    # (S, D, B)
        full, rem = d // 128, d % 128
        for t_ in range(full):
            x_T[:, g0 + t_, :, :] = xt[:, t_ * 128:(t_ + 1) * 128, :]
        if rem:
            x_T[:, g0 + full, :rem, :] = xt[:, full * 128:, :]
        ot, orow = ONES_POS[tw]
        x_T[:, g0 + ot, orow, :] = 1.0
    x_T = _to_bf16(x_T)

    def cell_params(tw, l, d_):
        return params[tw][l][d_]   # (W_ih, W_hh, b_ih, b_hh)

    in_maps = []
    for k in range(NC_):
        wsh = np.zeros((NWSH, 128, 512), np.float32)
        for tw, l in PAIRS:
            Wf = cell_params(tw, l, 0)
            Wb = cell_params(tw, l, 1)
            cols_f = _gate_rows(np.asarray(Wf[0], np.float32), k)   # (in, 256)
            cols_b = _gate_rows(np.asarray(Wb[0], np.float32), k)
            bias = np.concatenate([_gate_bias(Wf[2], Wf[3], k),
                                   _gate_bias(Wb[2], Wb[3], k)])     # (512,)
            cols = np.concatenate([cols_f, cols_b], axis=1)          # (in, 512)
            o = WSH_OFF[(tw, l)]
            if l == 0:
                d = DX[tw]
                for t_ in range(XT_TILES[tw]):
                    lo = t_ * 128
                    hi = min(d, lo + 128)
                    if hi > lo:
                        wsh[o["x"] + t_, 0:hi - lo, :] = cols[lo:hi, :]
                ot, orow = ONES_POS[tw]
                wsh[o["x"] + ot, orow, :] = bias
                for fi, key in ((0, "f1"), (1, "f2")):
                    base = d + fi * 512
                    for t_ in range(HKT):
                        wsh[o[key] + t_, :, :] = cols[base + t_ * 128: base + (t_ + 1) * 128, :]
            else:
                for t_ in range(HKT):
                    wsh[o["hf"] + t_, :, :] = cols[t_ * 128:(t_ + 1) * 128, :]
                    wsh[o["hb"] + t_, :, :] = cols[512 + t_ * 128: 512 + (t_ + 1) * 128, :]
                wsh[o["bias"], L1_BIAS_ROW, :] = bias
        whh = np.zeros((12, HKT, 128, 256), np.float32)
        for ci, (tw, l, d_) in enumerate(CELLS):
            Wc = cell_params(tw, l, d_)
            cols = _gate_rows(np.asarray(Wc[1], np.float32), k)      # (512, 256)
            for t_ in range(HKT):
                whh[ci, t_, :, :] = cols[t_ * 128:(t_ + 1) * 128, :]
        in_maps.append({"x_T": x_T, "wsh": _to_bf16(wsh), "whh": _to_bf16(whh)})
    return in_maps


def host_decode(results, dec_W, dec_b, n_steps=S):
    """results: list of 8 per-core dicts with 'st_out' (3, 128, 64) f32."""
    hidden = np.zeros((B, 3 * H), np.float32)
    for k in range(NC_):
        st = np.asarray(results[k]["st_out"], np.float32)   # (3, B, 64)
        for ti in range(3):
            hidden[:, ti * H + k * CH: ti * H + (k + 1) * CH] = st[ti]
    hidden *= 1.0 / n_steps
    dec_W = np.asarray(dec_W, np.float32)
    dec_b = np.asarray(dec_b, np.float32)
    return hidden @ dec_W.T + dec_b


# ---------------- bass kernel builder ----------------

def build_kernel(n_steps=S, cc_mode="real", with_ew=True, nd=NC_):
    nc = bacc.Bacc("TRN2", target_bir_lowering=False, debug=False,
                   enable_asserts=False, num_devices=nd)
    x_dram = nc.dram_tensor("x_T", (n_steps, NXT, 128, 128), BF16,
                            kind="ExternalInput").ap()
    wsh_dram = nc.dram_tensor("wsh", (NWSH, 128, 512), BF16,
                              kind="ExternalInput").ap()
    whh_dram = nc.dram_tensor("whh", (12, HKT, 128, 256), BF16,
                              kind="ExternalInput").ap()
    st_dram = nc.dram_tensor("st_out", (3, 128, CH), F32,
                             kind="ExternalOutput").ap()

    RG = [list(range(NC_))]

    with tile.TileContext(nc) as tc:
        with contextlib.ExitStack() as ctx:
            persist = ctx.enter_context(tc.tile_pool(name="persist", bufs=1))
            xp = ctx.enter_context(tc.tile_pool(name="xp", bufs=2))
            hg = ctx.enter_context(tc.tile_pool(name="hg", bufs=2))
            actp = ctx.enter_context(tc.tile_pool(name="actp", bufs=2))
            tmpp = ctx.enter_context(tc.tile_pool(name="tmpp", bufs=4))
            cp = ctx.enter_context(tc.tile_pool(name="cp", bufs=2))
            hp = ctx.enter_context(tc.tile_pool(name="hp", bufs=2))
            htp = ctx.enter_context(tc.tile_pool(name="htp", bufs=2))
            stp = ctx.enter_context(tc.tile_pool(name="stp", bufs=2))
            pg = ctx.enter_context(tc.tile_pool(name="pg", bufs=1, space="PSUM"))
            ptr = ctx.enter_context(tc.tile_pool(name="ptr", bufs=2, space="PSUM"))
            dramp = ctx.enter_context(tc.tile_pool(name="dramp", bufs=2, space="DRAM"))

            # ---- weight preload
            wsh_sb = persist.tile([128, NWSH, 512], BF16)
            nc.sync.dma_start(wsh_sb[:], wsh_dram.rearrange("w p n -> p w n"))
            whh_sb = persist.tile([128, 12 * HKT, 256], BF16)
            nc.sync.dma_start(whh_sb[:], whh_dram.rearrange("c k p n -> p (c k) n"))

            # persistent state handles, rotated via pool tags
            c_tiles = {}    # pair -> tile (128, 128) f32  [fwd 64 | bwd 64]
            st_tiles = {}   # tower-idx -> tile (128, 64) f32
            hl_all = {}     # layer -> gathered-h tile (128, 2, 3, HKT, 128) bf16
            hl_tiles = {}   # (pair, dir) -> (tile, j) accessor base

            ident = persist.tile([128, 128], BF16)
            from concourse.masks import make_identity
            make_identity(nc, ident[:])

            def load_x(s):
                t_ = xp.tile([128, NXT, 128], BF16, tag="x")
                nc.scalar.dma_start(t_[:], x_dram[s].rearrange("g p b -> p g b"))
                return t_

            def pair_idx(tw, l):
                return PAIRS.index((tw, l))

            def mm(gt_ap, lhsT, rhs, start, stop):
                nc.tensor.matmul(gt_ap, lhsT, rhs, start=start, stop=stop)

            def emit_shared(gt, tw, l, x_sb_, first, last):
                """Shared-K matmuls (x part for l0 / h_l0 part + bias for l1)."""
                o = WSH_OFF[(tw, l)]
                if l == 0:
                    g0 = XT_OFF[tw]
                    nx = XT_TILES[tw]
                    for t_ in range(nx):
                        mm(gt[:], x_sb_[:, g0 + t_, :], wsh_sb[:, o["x"] + t_, :],
                           start=first and t_ == 0, stop=last and t_ == nx - 1)
                else:
                    for d_, key in ((0, "hf"), (1, "hb")):
                        src, j = hl_tiles[(pair_idx(tw, 0), d_)]
                        for t_ in range(HKT):
                            mm(gt[:], src[:, t_, d_, j, :], wsh_sb[:, o[key] + t_, :],
                               start=first and d_ == 0 and t_ == 0, stop=False)
                    mm(gt[:], x_sb_[:, L1_BIAS_TILE, :], wsh_sb[:, o["bias"], :],
                       start=False, stop=last)

            def emit_feeds(gt, tw, last):
                o = WSH_OFF[(tw, 0)]
                f1, f2 = FEEDS[tw]
                for ki, (key, ftw) in enumerate((("f1", f1), ("f2", f2))):
                    src, j = hl_tiles[(pair_idx(ftw, 1), 1)]  # l1 bwd of feed tower
                    for t_ in range(HKT):
                        mm(gt[:], src[:, t_, 1, j, :], wsh_sb[:, o[key] + t_, :],
                           start=False, stop=last and ki == 1 and t_ == HKT - 1)

            def emit_whh(gt, tw, l, first):
                p = pair_idx(tw, l)
                for d_ in (0, 1):
                    ci = p * 2 + d_
                    src, j = hl_tiles[(p, d_)]
                    for t_ in range(HKT):
                        mm(gt[:, d_ * 256:(d_ + 1) * 256], src[:, t_, d_, j, :],
                           whh_sb[:, ci * HKT + t_, :],
                           start=first and d_ == 0 and t_ == 0, stop=False)

            def emit_elementwise(s, tw, l, gt):
                """gates (128, 512) psum [fwd ifog | bwd ifog] -> h_pair bf16;
                updates c and (for l1) st."""
                p = pair_idx(tw, l)
                if not with_ew:
                    h_pair = hp.tile([128, 128], BF16, tag=f"h{p}")
                    nc.vector.tensor_copy(h_pair[:], gt[:, 0:128])
                    if l == 1:
                        ti = TOWERS.index(tw)
                        st_new = stp.tile([128, 64], F32, tag=f"st{ti}")
                        nc.vector.tensor_copy(st_new[:], gt[:, 128:192])
                        st_tiles[ti] = st_new
                    return h_pair
                acts = actp.tile([128, 512], F32, tag=f"acts{p}")
                gtv = gt[:].rearrange("p (d n) -> p d n", d=2)
                av = acts[:].rearrange("p (d n) -> p d n", d=2)
                nc.scalar.activation(av[:, :, 0:192], gtv[:, :, 0:192],
                                     mybir.ActivationFunctionType.Sigmoid)
                nc.scalar.activation(av[:, :, 192:256], gtv[:, :, 192:256],
                                     mybir.ActivationFunctionType.Tanh)
                i_ap = av[:, :, 0:64]
                f_ap = av[:, :, 64:128]
                o_ap = av[:, :, 128:192]
                g_ap = av[:, :, 192:256]
                ig = tmpp.tile([128, 128], F32, tag="ig")
                igv = ig[:].rearrange("p (d n) -> p d n", d=2)
                nc.vector.tensor_mul(igv, i_ap, g_ap)
                c_new = cp.tile([128, 128], F32, tag=f"c{p}")
                cv = c_new[:].rearrange("p (d n) -> p d n", d=2)
                if s == 0:
                    nc.vector.tensor_copy(c_new[:], ig[:])
                else:
                    c_old = c_tiles[p]
                    nc.vector.tensor_mul(cv, f_ap,
                                         c_old[:].rearrange("p (d n) -> p d n", d=2))
                    nc.vector.tensor_add(c_new[:], c_new[:], ig[:])
                c_tiles[p] = c_new
                tanh_c = tmpp.tile([128, 128], F32, tag="tanh_c")
                nc.scalar.activation(tanh_c[:], c_new[:],
                                     mybir.ActivationFunctionType.Tanh)
                h_pair = hp.tile([128, 128], BF16, tag=f"h{p}")
                nc.vector.tensor_mul(h_pair[:].rearrange("p (d n) -> p d n", d=2),
                                     o_ap, tanh_c[:].rearrange("p (d n) -> p d n", d=2))
                if l == 1:
                    # st += h_bwd chunk, in fp32 (recompute the bwd mul in f32)
                    ti = TOWERS.index(tw)
                    h64 = tmpp.tile([128, 64], F32, tag="h64")
                    nc.vector.tensor_mul(h64[:], av[:, 1, 128:192],
                                         tanh_c[:, 64:128])
                    st_new = stp.tile([128, 64], F32, tag=f"st{ti}")
                    if s == 0:
                        nc.vector.tensor_copy(st_new[:], h64[:])
                    else:
                        nc.vector.tensor_add(st_new[:], st_tiles[ti][:], h64[:])
                    st_tiles[ti] = st_new
                return h_pair

            def emit_gather(l, h_pairs):
                """Transpose 3 h_pair tiles, AllGather, land per-(pair,dir)
                stationary tiles into hl_tiles."""
                ht = htp.tile([128, 3, 128], BF16, tag=f"ht{l}")
                for j in range(3):
                    tr = ptr.tile([128, 128], BF16, tag="tr")
                    nc.tensor.transpose(tr[:], h_pairs[j][:], ident[:])
                    nc.vector.tensor_copy(ht[:, j, :], tr[:])
                # agin rows land in ht's row order: row p = (dir, sub), so the
                # gathered buffer is (rank, d, sub, pair, b) — readback needs
                # only 3-dim APs then.
                agin = dramp.tile([128, 3, 128], BF16, tag=f"agin{l}")
                nc.sync.dma_start(agin[:], ht[:])
                agout = dramp.tile([NC_, 128, 3, 128], BF16, tag=f"agout{l}")
                if cc_mode == "real":
                    nc.gpsimd.collective_compute(
                        "AllGather", mybir.AluOpType.bypass, replica_groups=RG,
                        ins=[agin.opt()], outs=[agout.opt()])
                else:
                    for r in range(NC_):
                        nc.sync.dma_start(agout[r], agin[:])
                hla = hg.tile([128, HKT, 2, 3, 128], BF16, tag=f"hla{l}")
                # gathered rank r = 2*kt + phi owns feature rows
                # [128*kt + 64*phi, +64) of each (pair, dir); view the bounce
                # through the tile AP so Tile tracks the read.
                agv = agout[:].rearrange("(kt phi) (dd sub) g b -> phi dd sub kt (g b)",
                                         phi=2, dd=2)
                for phi in (0, 1):
                    for d_ in (0, 1):
                        eng = nc.sync if d_ == 0 else nc.scalar
                        eng.dma_start(
                            hla[64 * phi:64 * phi + 64, :, d_, :, :],
                            agv[phi, d_])
                hl_all[l] = hla
                for j in range(3):
                    p = 3 * l + j
                    for d_ in (0, 1):
                        hl_tiles[(p, d_)] = (hla, j)

            # ---------------- main loop ----------------
            x_sb = load_x(0)
            for s in range(n_steps):
                x_next = load_x(s + 1) if s + 1 < n_steps else None
                # ---- layer 0: x-part + recurrent first (ready during AG2(s-1)),
                # cross-feeds last (they wait on AG2(s-1)).
                gts = {}
                for tw in TOWERS:
                    gt = pg.tile([128, 512], F32, tag=f"g{pair_idx(tw, 0)}")
                    gts[tw] = gt
                    emit_shared(gt, tw, 0, x_sb, first=True, last=(s == 0))
                    if s > 0:
                        emit_whh(gt, tw, 0, first=False)
                if s > 0:
                    for tw in TOWERS:
                        emit_feeds(gts[tw], tw, last=True)
                h_l0 = [emit_elementwise(s, tw, 0, gts[tw]) for tw in TOWERS]
                # ---- layer 1 recurrent part first: depends only on AG2(s-1),
                # so PE chews it while the l0 elementwise + AG1 are in flight.
                gts1 = {}
                for tw in TOWERS:
                    gt = pg.tile([128, 512], F32, tag=f"g{pair_idx(tw, 1)}")
                    gts1[tw] = gt
                    if s > 0:
                        emit_whh(gt, tw, 1, first=True)
                emit_gather(0, h_l0)
                for tw in TOWERS:
                    emit_shared(gts1[tw], tw, 1, x_sb, first=(s == 0), last=True)
                h_l1 = [emit_elementwise(s, tw, 1, gts1[tw]) for tw in TOWERS]
                if s + 1 < n_steps:
                    emit_gather(1, h_l1)
                if x_next is not None:
                    x_sb = x_next

            # ---- write out st sums
            for ti in range(3):
                nc.sync.dma_start(st_dram[ti], st_tiles[ti][:])

    nc.compile()
    return nc


# ---------------- public entry ----------------
_CACHED = {}


def _get_kernel(n_steps=S):
    if n_steps not in _CACHED:
        _CACHED[n_steps] = build_kernel(n_steps)
    return _CACHED[n_steps]


def run_on_hw(inputs, n_steps=S):
    nc = _get_kernel(n_steps)
    in_maps = prep_in_maps(**inputs, n_steps=n_steps)
    res = run_bass_kernel_spmd(nc, in_maps, core_ids=list(range(NC_)))
    return host_decode(res.results, inputs["dec_W"], inputs["dec_b"],
                       n_steps=n_steps).astype(np.float32)


def kernel(t_input, v_input, a_input, params_t, params_v, params_a, dec_W, dec_b):
    return run_on_hw(dict(t_input=t_input, v_input=v_input, a_input=a_input,
                          params_t=params_t, params_v=params_v, params_a=params_a,
                          dec_W=dec_W, dec_b=dec_b), n_steps=S)
